# revision 3
# baseline (speedup 1.0000x reference)
"""Trainium2 Bass kernel v2 for nn_CausalFlowModel (LSTM flow model).

Per core (bc=256 batch cols, sorted ascending by h_len, split into two
interleaved halves A/B of 128):
  - HXRING [82, 24*256] fp16: slot(t)=t%24 holds [h_{t-1}(72); x_t(9); 1].
    x loaded 12 steps per DMA; h written in place by the cell update.
  - Per step, per half: 4 matmuls (K=82, M=72, N=live) -> psum gp[72,4,128],
    one fused sigmoid over all four gates (g pre-scaled x2 so
    tanh(g)=2*sigmoid(2g)-1), elementwise tail on DVE (half A) / GpSimd
    (half B).
  - Ragged h[l-1]/h[l-2] captures: per-step is_equal masks over small
    static windows + copy_predicated into H1/H2 (no DRAM, no registers).
  - Live-window shrink: ops cover only columns whose l has not expired
    (static per-step slice bounds from min over cores).
"""

import numpy as np

import concourse.bacc as bacc
import concourse.bass as bass
import concourse.mybir as mybir
import concourse.tile as tile
from concourse.bass_utils import run_bass_kernel_spmd

F32 = mybir.dt.float32
F32R = mybir.dt.float32r
F16 = mybir.dt.float16
AF = mybir.ActivationFunctionType
ALU = mybir.AluOpType

B, T, SD, CD = 2048, 512, 8, 8
H = 72
KG = H + CD + 2          # 82 = h(72) + x(9) + ones(1)
NCORES = 8
BC = B // NCORES         # 256
HB = BC // 2             # 128 cols per half
NSLOT = 24               # ring slots
CHUNK = 12               # steps per x DMA


class Cfg:
    def __init__(self):
        self.mm_dt = F16
        self.tstop = T


# --------------------------------------------------------------------------- #
# host-side preparation
# --------------------------------------------------------------------------- #

def _gate_reorder(w4h: np.ndarray) -> np.ndarray:
    i, f, g, o = np.split(w4h, 4, axis=0)
    return np.concatenate([f, i, g, o], axis=0)


def host_prep(inputs: dict, cfg: Cfg):
    lens = np.asarray(inputs["h_lens"]).astype(np.int64)
    order = np.argsort(lens, kind="stable")

    W_ih = np.asarray(inputs["W_ih"], np.float32)
    W_hh = np.asarray(inputs["W_hh"], np.float32)
    b_g = np.asarray(inputs["b_ih"], np.float32) + np.asarray(inputs["b_hh"], np.float32)
    Wg = np.concatenate([W_hh, W_ih, b_g[:, None]], axis=1)   # [288, 82]
    Wg = _gate_reorder(Wg)                                    # (f,i,g,o)
    Wg[2 * H:3 * H] *= 2.0                                    # sigma-trick on g
    wg_all = np.ascontiguousarray(Wg.T)                       # [82, 288]

    def kchunks(wT, chunk=128):
        return [np.ascontiguousarray(wT[s:s + chunk])
                for s in range(0, wT.shape[0], chunk)]

    we1T = np.ascontiguousarray(np.asarray(inputs["enc_W1"], np.float32).T)
    we2T = np.ascontiguousarray(np.asarray(inputs["enc_W2"], np.float32).T)
    we3T = np.ascontiguousarray(np.asarray(inputs["enc_W3"], np.float32).T)
    wd1T = np.ascontiguousarray(np.asarray(inputs["dec_W1"], np.float32).T)
    wd2T = np.ascontiguousarray(np.asarray(inputs["dec_W2"], np.float32).T)
    wd3T = np.ascontiguousarray(np.asarray(inputs["dec_W3"], np.float32).T)

    def bias_cols(b, p=128):
        ncol = (len(b) + p - 1) // p
        out = np.zeros((p, ncol), np.float32)
        for c in range(ncol):
            seg = b[c * p:(c + 1) * p]
            out[: len(seg), c] = seg
        return out

    shared = {
        "wg_all": wg_all.astype(np.float16),
        "we1T": we1T,
        "we2Tk0": kchunks(we2T)[0], "we2Tk1": kchunks(we2T)[1],
        "we3Tk0": kchunks(we3T)[0], "we3Tk1": kchunks(we3T)[1],
        "wd1T": wd1T,
        "wd2Tk0": kchunks(wd2T)[0], "wd2Tk1": kchunks(wd2T)[1],
        "wd2Tk2": kchunks(wd2T)[2],
        "wd3Tk0": kchunks(wd3T)[0], "wd3Tk1": kchunks(wd3T)[1],
        "wd3Tk2": kchunks(wd3T)[2],
        "be1": bias_cols(np.asarray(inputs["enc_b1"], np.float32)),
        "be2": bias_cols(np.asarray(inputs["enc_b2"], np.float32)),
        "be3": bias_cols(np.asarray(inputs["enc_b3"], np.float32), p=64),
        "bd1": bias_cols(np.asarray(inputs["dec_b1"], np.float32)),
        "bd2": bias_cols(np.asarray(inputs["dec_b2"], np.float32)),
        "bd3": bias_cols(np.asarray(inputs["dec_b3"], np.float32), p=8),
    }

    x = np.asarray(inputs["x"], np.float32)
    rnn = np.asarray(inputs["rnn_input"], np.float32)
    deltas = np.asarray(inputs["deltas"], np.float32)

    in_maps, perms = [], []
    lens_hk = np.zeros((NCORES, 2, HB), np.int64)
    for k in range(NCORES):
        perm = order[np.arange(BC) * NCORES + k]       # sorted ascending
        colperm = np.concatenate([perm[0::2], perm[1::2]])
        perms.append(colperm)
        lk = lens[colperm]                             # [256]
        lens_hk[k, 0] = lk[:HB]
        lens_hk[k, 1] = lk[HB:]
        dsel = deltas[colperm, lk - 1, 0].astype(np.float32)   # [256]
        d1 = np.broadcast_to(dsel, (H, BC)).copy()
        d2 = np.broadcast_to(1.0 - dsel, (H, BC)).copy()
        LENS = np.broadcast_to(lk.astype(np.float32), (H, BC)).copy()
        # XD: [10, T*256], col t*256+j = [rnn[colperm[j], t, :]; 1.0]
        rk = rnn[colperm].transpose(2, 1, 0).reshape(CD + 1, T * BC)
        rk = np.concatenate([rk, np.ones((1, T * BC), np.float32)], axis=0)
        m = dict(shared)
        m.update({
            "xTr": np.ascontiguousarray(x[colperm].T),                 # [8,256] f32
            "xTh": np.ascontiguousarray(x[colperm].T).astype(np.float16),
            "XD": np.ascontiguousarray(rk).astype(np.float16),
            "d1": d1, "d2": d2, "LENS": LENS,
        })
        in_maps.append(m)

    # static live windows / capture windows (shared across cores: min/max)
    tvals = np.arange(T + 3)
    lo = np.zeros((2, T + 3), np.int64)       # live start = min_k #{l <= t}
    wlo = np.full((2, T + 3), HB, np.int64)   # capture window per value v
    whi = np.zeros((2, T + 3), np.int64)
    for h in range(2):
        for k in range(NCORES):
            lk = np.sort(lens_hk[k, h])
            cnt_le = np.searchsorted(lk, tvals, side="right")
            cnt_lt = np.searchsorted(lk, tvals, side="left")
            if k == 0:
                lo[h] = cnt_le
            else:
                lo[h] = np.minimum(lo[h], cnt_le)
            wlo[h] = np.minimum(wlo[h], cnt_lt)
            whi[h] = np.maximum(whi[h], cnt_le)
    meta = {
        "lo": lo, "wlo": wlo, "whi": whi,
        "lmin": int(lens.min()), "lmax": int(lens.max()),
    }
    return in_maps, perms, meta


# --------------------------------------------------------------------------- #
# device kernel
# --------------------------------------------------------------------------- #

def build_nc(cfg: Cfg, meta):
    nc = bacc.Bacc("TRN2", target_bir_lowering=False, debug=False,
                   enable_asserts=False, num_devices=NCORES)
    RD = cfg.mm_dt
    lo_t, wlo_t, whi_t = meta["lo"], meta["wlo"], meta["whi"]
    lmin = meta["lmin"]

    def din(name, shape, dt=F32):
        return nc.dram_tensor(name, list(shape), dt, kind="ExternalInput").ap()

    ins = {
        "xTr": din("xTr", [SD, BC], F32R),
        "xTh": din("xTh", [SD, BC], RD),
        "XD": din("XD", [CD + 2, T * BC], RD),
        "d1": din("d1", [H, BC]), "d2": din("d2", [H, BC]),
        "LENS": din("LENS", [H, BC]),
        "wg_all": din("wg_all", [KG, 4 * H], RD),
        "we1T": din("we1T", [SD, 256], F32R),
        "we2Tk0": din("we2Tk0", [128, 256], F32R), "we2Tk1": din("we2Tk1", [128, 256], F32R),
        "we3Tk0": din("we3Tk0", [128, 64], F32R), "we3Tk1": din("we3Tk1", [128, 64], F32R),
        "wd1T": din("wd1T", [H, 288], F32R),
        "wd2Tk0": din("wd2Tk0", [128, 288], F32R), "wd2Tk1": din("wd2Tk1", [128, 288], F32R),
        "wd2Tk2": din("wd2Tk2", [32, 288], F32R),
        "wd3Tk0": din("wd3Tk0", [128, SD], F32R), "wd3Tk1": din("wd3Tk1", [128, SD], F32R),
        "wd3Tk2": din("wd3Tk2", [32, SD], F32R),
        "be1": din("be1", [128, 2]), "be2": din("be2", [128, 2]),
        "be3": din("be3", [64, 1]),
        "bd1": din("bd1", [128, 3]), "bd2": din("bd2", [128, 3]),
        "bd3": din("bd3", [SD, 1]),
    }
    out_dram = nc.dram_tensor("out", [SD, BC], F32, kind="ExternalOutput").ap()

    with tile.TileContext(nc) as tc:
        with tc.tile_pool(name="const", bufs=1) as cpool, \
             tc.tile_pool(name="work", bufs=2) as wpool, \
             tc.tile_pool(name="psum", bufs=2, space="PSUM") as ppool:

            sb = {}
            for name in ["wg_all", "we1T", "we2Tk0", "we2Tk1", "we3Tk0",
                         "we3Tk1", "wd1T", "wd2Tk0", "wd2Tk1", "wd2Tk2",
                         "wd3Tk0", "wd3Tk1", "wd3Tk2", "be1", "be2", "be3",
                         "bd1", "bd2", "bd3", "d1", "d2", "LENS"]:
                ap = ins[name]
                t_ = cpool.tile(list(ap.shape), ap.dtype, name=f"sb_{name}")
                nc.sync.dma_start(t_, ap)
                sb[name] = t_

            # persistent state
            HX = cpool.tile([KG, NSLOT * BC], RD, name="HX")
            CTa = cpool.tile([H, 2, HB], RD, name="CTa")   # [c | tg] half A
            CTb = cpool.tile([H, 2, HB], RD, name="CTb")
            H1 = cpool.tile([H, BC], RD, name="H1")
            H2 = cpool.tile([H, BC], RD, name="H2")
            I32 = mybir.dt.int32
            Ma = [cpool.tile([H, HB], I32, name=f"Ma{r}") for r in range(2)]
            Mb = [cpool.tile([H, HB], I32, name=f"Mb{r}") for r in range(2)]
            nc.vector.memset(CTa, 0.0)
            nc.gpsimd.memset(CTb, 0.0)
            nc.vector.memset(H1, 0.0)
            nc.vector.memset(H2, 0.0)
            nc.gpsimd.memset(HX.bitcast(mybir.dt.uint16), 0)

            # x chunks 0 and 1 (steps 0..23); row KG-1 gets the ones row
            for b_ in range(2):
                nc.sync.dma_start(
                    HX[H:KG, b_ * CHUNK * BC:(b_ + 1) * CHUNK * BC],
                    ins["XD"][:, b_ * CHUNK * BC:(b_ + 1) * CHUNK * BC])

            def mm(out, lhsT, rhs, start=True, stop=True):
                nc.tensor.matmul(out, lhsT, rhs, start=start, stop=stop)

            # ---- encoder MLP -> h0 into slot 0 ----------------------------
            nc.sync.dma_start(HX[0:SD, 0:BC], ins["xTh"])
            ex = wpool.tile([SD, BC], F32R, name="ex")
            nc.sync.dma_start(ex, ins["xTr"])
            ez1p = ppool.tile([128, 512], F32, name="ez1p", tag="ps")
            for c in range(2):
                mm(ez1p[:, 256 * c:256 * (c + 1)],
                   sb["we1T"][:, 128 * c:128 * (c + 1)], ex)
            ez1 = wpool.tile([128, 512], F32R, name="ez1")
            for c in range(2):
                nc.scalar.activation(ez1[:, 256 * c:256 * (c + 1)],
                                     ez1p[:, 256 * c:256 * (c + 1)],
                                     AF.Tanh, bias=sb["be1"][:, c:c + 1])
            ez2p = ppool.tile([128, 512], F32, name="ez2p", tag="ps")
            for c in range(2):
                for k in range(2):
                    mm(ez2p[:, 256 * c:256 * (c + 1)],
                       sb[f"we2Tk{k}"][:, 128 * c:128 * (c + 1)],
                       ez1[:, 256 * k:256 * (k + 1)],
                       start=(k == 0), stop=(k == 1))
            ez2 = wpool.tile([128, 512], F32R, name="ez2")
            for c in range(2):
                nc.scalar.activation(ez2[:, 256 * c:256 * (c + 1)],
                                     ez2p[:, 256 * c:256 * (c + 1)],
                                     AF.Tanh, bias=sb["be2"][:, c:c + 1])
            eh0p = ppool.tile([64, 256], F32, name="eh0p", tag="ps")
            for k in range(2):
                mm(eh0p, sb[f"we3Tk{k}"], ez2[:, 256 * k:256 * (k + 1)],
                   start=(k == 0), stop=(k == 1))
            eh0 = wpool.tile([64, 256], RD, name="eh0")
            nc.scalar.activation(eh0, eh0p, AF.Identity, bias=sb["be3"][:, 0:1])
            nc.sync.dma_start(HX[SD:H, 0:BC], eh0)

            # ---- LSTM over T steps ----------------------------------------
            for t in range(cfg.tstop):
                base = (t % NSLOT) * BC
                nbase = ((t + 1) % NSLOT) * BC
                loA = int(lo_t[0][min(t, T)])
                loB = int(lo_t[1][min(t, T)])
                lvA = HB - loA
                lvB = HB - loB
                if lvA <= 0 and lvB <= 0:
                    break
                if t % CHUNK == 0 and t > 0:
                    b_ = t // CHUNK + 1
                    if b_ * CHUNK < T:
                        c0 = (b_ * CHUNK % NSLOT) * BC
                        n_ = min(CHUNK, T - b_ * CHUNK)
                        nc.sync.dma_start(
                            HX[H:KG, c0:c0 + n_ * BC],
                            ins["XD"][:, b_ * CHUNK * BC:(b_ * CHUNK + n_) * BC])

                # ---- half A: matmuls + ACT + DVE tail ---------------------
                if lvA > 0:
                    rhsA = HX[0:KG, base + loA:base + HB]
                    gpA = ppool.tile([H, 4, HB], F32, name="gpA", tag="psA")
                    for c in range(4):
                        mm(gpA[:, c, loA:HB], sb["wg_all"][:, H * c:H * (c + 1)], rhsA)
                if lvB > 0:
                    rhsB = HX[0:KG, base + HB + loB:base + 2 * HB]
                    gpB = ppool.tile([H, 4, HB], F32, name="gpB", tag="psB")
                    for c in range(4):
                        mm(gpB[:, c, loB:HB], sb["wg_all"][:, H * c:H * (c + 1)], rhsB)

                # sigma split: F,I,G on the chain; O off-chain (h needs it late)
                if lvA > 0:
                    SA = wpool.tile([H, 4, HB], RD, name="SA")
                    nc.scalar.activation(SA[:, 0:3, loA:HB], gpA[:, 0:3, loA:HB], AF.Sigmoid)
                if lvB > 0:
                    SB = wpool.tile([H, 4, HB], RD, name="SB")
                    nc.scalar.activation(SB[:, 0:3, loB:HB], gpB[:, 0:3, loB:HB], AF.Sigmoid)
                if lvA > 0:
                    nc.scalar.activation(SA[:, 3, loA:HB], gpA[:, 3, loA:HB], AF.Sigmoid)
                if lvB > 0:
                    nc.scalar.activation(SB[:, 3, loB:HB], gpB[:, 3, loB:HB], AF.Sigmoid)

                # DVE queue: [A: tg,Tt,add] [B: tg,Tt,add] [hA] [hB] so the
                # other half's ops fill the tanh-wait hole.
                if lvA > 0:
                    nc.vector.tensor_scalar(CTa[:, 1, loA:HB], SA[:, 2, loA:HB],
                                            2.0, -1.0, op0=ALU.mult, op1=ALU.add)
                    TtA = wpool.tile([H, 2, HB], RD, name="TtA")
                    nc.vector.tensor_tensor(TtA[:, :, loA:HB], SA[:, 0:2, loA:HB],
                                            CTa[:, :, loA:HB], op=ALU.mult)
                    nc.vector.tensor_tensor(CTa[:, 0, loA:HB], TtA[:, 0, loA:HB],
                                            TtA[:, 1, loA:HB], op=ALU.add)
                if lvB > 0:
                    nc.vector.tensor_scalar(CTb[:, 1, loB:HB], SB[:, 2, loB:HB],
                                            2.0, -1.0, op0=ALU.mult, op1=ALU.add)
                    TtB = wpool.tile([H, 2, HB], RD, name="TtB")
                    nc.vector.tensor_tensor(TtB[:, :, loB:HB], SB[:, 0:2, loB:HB],
                                            CTb[:, :, loB:HB], op=ALU.mult)
                    nc.vector.tensor_tensor(CTb[:, 0, loB:HB], TtB[:, 0, loB:HB],
                                            TtB[:, 1, loB:HB], op=ALU.add)
                if lvA > 0:
                    TCa = wpool.tile([H, HB], RD, name="TCa")
                    nc.scalar.activation(TCa[:, loA:HB], CTa[:, 0, loA:HB], AF.Tanh)
                    nc.vector.tensor_tensor(HX[0:H, nbase + loA:nbase + HB],
                                            SA[:, 3, loA:HB], TCa[:, loA:HB],
                                            op=ALU.mult)
                if lvB > 0:
                    TCb = wpool.tile([H, HB], RD, name="TCb")
                    nc.scalar.activation(TCb[:, loB:HB], CTb[:, 0, loB:HB], AF.Tanh)
                    nc.vector.tensor_tensor(HX[0:H, nbase + HB + loB:nbase + 2 * HB],
                                            SB[:, 3, loB:HB], TCb[:, loB:HB],
                                            op=ALU.mult)

                # ---- captures (masks + predicated copies) -----------------
                if t >= lmin - 2:
                    for h_, (M, off) in enumerate([(Ma, 0), (Mb, HB)]):
                        v2 = t + 2
                        if v2 <= T and whi_t[h_][v2] > wlo_t[h_][v2]:
                            a, b2 = int(wlo_t[h_][v2]), int(whi_t[h_][v2])
                            eng = nc.gpsimd
                            eng.tensor_scalar(M[t % 2][:, a:b2],
                                              sb["LENS"][:, off + a:off + b2],
                                              float(v2), None, op0=ALU.is_equal)
                            nc.vector.copy_predicated(
                                H2[:, off + a:off + b2], M[t % 2][:, a:b2],
                                HX[0:H, nbase + off + a:nbase + off + b2])
                        v1 = t + 1
                        if v1 >= lmin and whi_t[h_][v1] > wlo_t[h_][v1]:
                            a, b1 = int(wlo_t[h_][v1]), int(whi_t[h_][v1])
                            nc.vector.copy_predicated(
                                H1[:, off + a:off + b1], M[(t + 1) % 2][:, a:b1],
                                HX[0:H, nbase + off + a:nbase + off + b1])

            # ---- dec_in = d1*H1 + d2*H2 -----------------------------------
            U1 = wpool.tile([H, BC], F32, name="U1")
            nc.vector.tensor_tensor(U1, sb["d1"], H1, op=ALU.mult)
            U2 = wpool.tile([H, BC], F32, name="U2")
            nc.vector.tensor_tensor(U2, sb["d2"], H2, op=ALU.mult)
            DI = wpool.tile([H, BC], F32R, name="DI")
            nc.vector.tensor_tensor(DI, U1, U2, op=ALU.add)

            # ---- decoder MLP ----------------------------------------------
            CH1 = [(0, 128), (128, 128), (256, 32)]
            dz1p = ppool.tile([128, 768], F32, name="dz1p", tag="ps")
            for c, (off, m_) in enumerate(CH1):
                mm(dz1p[0:m_, 256 * c:256 * c + BC], sb["wd1T"][:, off:off + m_], DI)
            dz1 = wpool.tile([128, 768], F32R, name="dz1")
            for c, (off, m_) in enumerate(CH1):
                nc.scalar.activation(dz1[0:m_, 256 * c:256 * c + BC],
                                     dz1p[0:m_, 256 * c:256 * c + BC],
                                     AF.Tanh, bias=sb["bd1"][0:m_, c:c + 1])
            dz2p = ppool.tile([128, 768], F32, name="dz2p", tag="ps")
            for c, (off, m_) in enumerate(CH1):
                for k, (koff, km) in enumerate(CH1):
                    mm(dz2p[0:m_, 256 * c:256 * c + BC],
                       sb[f"wd2Tk{k}"][0:km, off:off + m_],
                       dz1[0:km, 256 * k:256 * k + BC],
                       start=(k == 0), stop=(k == 2))
            dz2 = wpool.tile([128, 768], F32R, name="dz2")
            for c, (off, m_) in enumerate(CH1):
                nc.scalar.activation(dz2[0:m_, 256 * c:256 * c + BC],
                                     dz2p[0:m_, 256 * c:256 * c + BC],
                                     AF.Tanh, bias=sb["bd2"][0:m_, c:c + 1])
            dz3p = ppool.tile([SD, 256], F32, name="dz3p", tag="ps")
            for k, (koff, km) in enumerate(CH1):
                mm(dz3p, sb[f"wd3Tk{k}"][0:km, :],
                   dz2[0:km, 256 * k:256 * k + BC],
                   start=(k == 0), stop=(k == 2))
            OUT = wpool.tile([SD, BC], F32, name="OUT")
            nc.scalar.activation(OUT, dz3p, AF.Identity, bias=sb["bd3"][:, 0:1])
            nc.sync.dma_start(out_dram, OUT)

            import os as _os
            if _os.environ.get("KDBG"):
                h1d = nc.dram_tensor("dbg_H1", [H, BC], RD, kind="ExternalOutput").ap()
                h2d = nc.dram_tensor("dbg_H2", [H, BC], RD, kind="ExternalOutput").ap()
                hxd = nc.dram_tensor("dbg_HX", [KG, NSLOT * BC], RD, kind="ExternalOutput").ap()
                did = nc.dram_tensor("dbg_DI", [H, BC], F32, kind="ExternalOutput").ap()
                nc.sync.dma_start(h1d, H1)
                nc.sync.dma_start(h2d, H2)
                nc.sync.dma_start(hxd, HX)
                nc.sync.dma_start(did, DI.bitcast(F32))

    nc.compile()
    return nc


# --------------------------------------------------------------------------- #
# entry point
# --------------------------------------------------------------------------- #

def kernel(**inputs) -> np.ndarray:
    cfg = Cfg()
    in_maps, perms, meta = host_prep(inputs, cfg)
    nc = build_nc(cfg, meta)
    res = run_bass_kernel_spmd(nc, in_maps, core_ids=list(range(NCORES)))
    out = np.empty((B, SD), np.float32)
    for k in range(NCORES):
        out[perms[k]] = res.results[k]["out"].T
    return out


# revision 4
# speedup vs baseline: 1.0250x; 1.0250x over previous
"""Trainium2 Bass kernel v2 for nn_CausalFlowModel (LSTM flow model).

Per core (bc=256 batch cols, sorted ascending by h_len, split into two
interleaved halves A/B of 128):
  - HXRING [82, 24*256] fp16: slot(t)=t%24 holds [h_{t-1}(72); x_t(9); 1].
    x loaded 12 steps per DMA; h written in place by the cell update.
  - Per step, per half: 4 matmuls (K=82, M=72, N=live) -> psum gp[72,4,128],
    one fused sigmoid over all four gates (g pre-scaled x2 so
    tanh(g)=2*sigmoid(2g)-1), elementwise tail on DVE (half A) / GpSimd
    (half B).
  - Ragged h[l-1]/h[l-2] captures: per-step is_equal masks over small
    static windows + copy_predicated into H1/H2 (no DRAM, no registers).
  - Live-window shrink: ops cover only columns whose l has not expired
    (static per-step slice bounds from min over cores).
"""

import numpy as np

import concourse.bacc as bacc
import concourse.bass as bass
import concourse.mybir as mybir
import concourse.tile as tile
from concourse.bass_utils import run_bass_kernel_spmd

F32 = mybir.dt.float32
F32R = mybir.dt.float32r
F16 = mybir.dt.float16
AF = mybir.ActivationFunctionType
ALU = mybir.AluOpType

B, T, SD, CD = 2048, 512, 8, 8
H = 72
KG = H + CD + 2          # 82 = h(72) + x(9) + ones(1)
NCORES = 8
BC = B // NCORES         # 256
HB = BC // 2             # 128 cols per half
NSLOT = 24               # ring slots
CHUNK = 12               # steps per x DMA


class Cfg:
    def __init__(self):
        self.mm_dt = F16
        self.tstop = T


# --------------------------------------------------------------------------- #
# host-side preparation
# --------------------------------------------------------------------------- #

def _gate_reorder(w4h: np.ndarray) -> np.ndarray:
    i, f, g, o = np.split(w4h, 4, axis=0)
    return np.concatenate([f, i, g, o], axis=0)


def host_prep(inputs: dict, cfg: Cfg):
    lens = np.asarray(inputs["h_lens"]).astype(np.int64)
    order = np.argsort(lens, kind="stable")

    W_ih = np.asarray(inputs["W_ih"], np.float32)
    W_hh = np.asarray(inputs["W_hh"], np.float32)
    b_g = np.asarray(inputs["b_ih"], np.float32) + np.asarray(inputs["b_hh"], np.float32)
    Wg = np.concatenate([W_hh, W_ih, b_g[:, None]], axis=1)   # [288, 82]
    Wg = _gate_reorder(Wg)                                    # (f,i,g,o)
    Wg[2 * H:3 * H] *= 2.0                                    # sigma-trick on g
    wg_all = np.ascontiguousarray(Wg.T)                       # [82, 288]

    def kchunks(wT, chunk=128):
        return [np.ascontiguousarray(wT[s:s + chunk])
                for s in range(0, wT.shape[0], chunk)]

    we1T = np.ascontiguousarray(np.asarray(inputs["enc_W1"], np.float32).T)
    we2T = np.ascontiguousarray(np.asarray(inputs["enc_W2"], np.float32).T)
    we3T = np.ascontiguousarray(np.asarray(inputs["enc_W3"], np.float32).T)
    wd1T = np.ascontiguousarray(np.asarray(inputs["dec_W1"], np.float32).T)
    wd2T = np.ascontiguousarray(np.asarray(inputs["dec_W2"], np.float32).T)
    wd3T = np.ascontiguousarray(np.asarray(inputs["dec_W3"], np.float32).T)

    def bias_cols(b, p=128):
        ncol = (len(b) + p - 1) // p
        out = np.zeros((p, ncol), np.float32)
        for c in range(ncol):
            seg = b[c * p:(c + 1) * p]
            out[: len(seg), c] = seg
        return out

    shared = {
        "wg_all": wg_all.astype(np.float16),
        "we1T": we1T,
        "we2Tk0": kchunks(we2T)[0], "we2Tk1": kchunks(we2T)[1],
        "we3Tk0": kchunks(we3T)[0], "we3Tk1": kchunks(we3T)[1],
        "wd1T": wd1T,
        "wd2Tk0": kchunks(wd2T)[0], "wd2Tk1": kchunks(wd2T)[1],
        "wd2Tk2": kchunks(wd2T)[2],
        "wd3Tk0": kchunks(wd3T)[0], "wd3Tk1": kchunks(wd3T)[1],
        "wd3Tk2": kchunks(wd3T)[2],
        "be1": bias_cols(np.asarray(inputs["enc_b1"], np.float32)),
        "be2": bias_cols(np.asarray(inputs["enc_b2"], np.float32)),
        "be3": bias_cols(np.asarray(inputs["enc_b3"], np.float32), p=64),
        "bd1": bias_cols(np.asarray(inputs["dec_b1"], np.float32)),
        "bd2": bias_cols(np.asarray(inputs["dec_b2"], np.float32)),
        "bd3": bias_cols(np.asarray(inputs["dec_b3"], np.float32), p=8),
    }

    x = np.asarray(inputs["x"], np.float32)
    rnn = np.asarray(inputs["rnn_input"], np.float32)
    deltas = np.asarray(inputs["deltas"], np.float32)

    in_maps, perms = [], []
    lens_hk = np.zeros((NCORES, 2, HB), np.int64)
    for k in range(NCORES):
        perm = order[np.arange(BC) * NCORES + k]       # sorted ascending
        colperm = np.concatenate([perm[0::2], perm[1::2]])
        perms.append(colperm)
        lk = lens[colperm]                             # [256]
        lens_hk[k, 0] = lk[:HB]
        lens_hk[k, 1] = lk[HB:]
        dsel = deltas[colperm, lk - 1, 0].astype(np.float32)   # [256]
        d1 = np.broadcast_to(dsel, (H, BC)).copy()
        d2 = np.broadcast_to(1.0 - dsel, (H, BC)).copy()
        LENS = np.broadcast_to(lk.astype(np.float32), (H, BC)).copy()
        # XD: [10, T*256], col t*256+j = [rnn[colperm[j], t, :]; 1.0]
        rk = rnn[colperm].transpose(2, 1, 0).reshape(CD + 1, T * BC)
        rk = np.concatenate([rk, np.ones((1, T * BC), np.float32)], axis=0)
        m = dict(shared)
        m.update({
            "xTr": np.ascontiguousarray(x[colperm].T),                 # [8,256] f32
            "xTh": np.ascontiguousarray(x[colperm].T).astype(np.float16),
            "XD": np.ascontiguousarray(rk).astype(np.float16),
            "d1": d1, "d2": d2, "LENS": LENS,
        })
        in_maps.append(m)

    # static live windows / capture windows (shared across cores: min/max)
    tvals = np.arange(T + 3)
    lo = np.zeros((2, T + 3), np.int64)       # live start = min_k #{l <= t}
    wlo = np.full((2, T + 3), HB, np.int64)   # capture window per value v
    whi = np.zeros((2, T + 3), np.int64)
    for h in range(2):
        for k in range(NCORES):
            lk = np.sort(lens_hk[k, h])
            cnt_le = np.searchsorted(lk, tvals, side="right")
            cnt_lt = np.searchsorted(lk, tvals, side="left")
            if k == 0:
                lo[h] = cnt_le
            else:
                lo[h] = np.minimum(lo[h], cnt_le)
            wlo[h] = np.minimum(wlo[h], cnt_lt)
            whi[h] = np.maximum(whi[h], cnt_le)
    meta = {
        "lo": lo, "wlo": wlo, "whi": whi,
        "lmin": int(lens.min()), "lmax": int(lens.max()),
    }
    return in_maps, perms, meta


# --------------------------------------------------------------------------- #
# device kernel
# --------------------------------------------------------------------------- #

def build_nc(cfg: Cfg, meta):
    nc = bacc.Bacc("TRN2", target_bir_lowering=False, debug=False,
                   enable_asserts=False, num_devices=NCORES)
    RD = cfg.mm_dt
    lo_t, wlo_t, whi_t = meta["lo"], meta["wlo"], meta["whi"]
    lmin = meta["lmin"]

    def din(name, shape, dt=F32):
        return nc.dram_tensor(name, list(shape), dt, kind="ExternalInput").ap()

    ins = {
        "xTr": din("xTr", [SD, BC], F32R),
        "xTh": din("xTh", [SD, BC], RD),
        "XD": din("XD", [CD + 2, T * BC], RD),
        "d1": din("d1", [H, BC]), "d2": din("d2", [H, BC]),
        "LENS": din("LENS", [H, BC]),
        "wg_all": din("wg_all", [KG, 4 * H], RD),
        "we1T": din("we1T", [SD, 256], F32R),
        "we2Tk0": din("we2Tk0", [128, 256], F32R), "we2Tk1": din("we2Tk1", [128, 256], F32R),
        "we3Tk0": din("we3Tk0", [128, 64], F32R), "we3Tk1": din("we3Tk1", [128, 64], F32R),
        "wd1T": din("wd1T", [H, 288], F32R),
        "wd2Tk0": din("wd2Tk0", [128, 288], F32R), "wd2Tk1": din("wd2Tk1", [128, 288], F32R),
        "wd2Tk2": din("wd2Tk2", [32, 288], F32R),
        "wd3Tk0": din("wd3Tk0", [128, SD], F32R), "wd3Tk1": din("wd3Tk1", [128, SD], F32R),
        "wd3Tk2": din("wd3Tk2", [32, SD], F32R),
        "be1": din("be1", [128, 2]), "be2": din("be2", [128, 2]),
        "be3": din("be3", [64, 1]),
        "bd1": din("bd1", [128, 3]), "bd2": din("bd2", [128, 3]),
        "bd3": din("bd3", [SD, 1]),
    }
    out_dram = nc.dram_tensor("out", [SD, BC], F32, kind="ExternalOutput").ap()

    with tile.TileContext(nc) as tc:
        with tc.tile_pool(name="const", bufs=1) as cpool, \
             tc.tile_pool(name="work", bufs=2) as wpool, \
             tc.tile_pool(name="psum", bufs=2, space="PSUM") as ppool:

            sb = {}
            for name in ["wg_all", "we1T", "we2Tk0", "we2Tk1", "we3Tk0",
                         "we3Tk1", "wd1T", "wd2Tk0", "wd2Tk1", "wd2Tk2",
                         "wd3Tk0", "wd3Tk1", "wd3Tk2", "be1", "be2", "be3",
                         "bd1", "bd2", "bd3", "d1", "d2", "LENS"]:
                ap = ins[name]
                t_ = cpool.tile(list(ap.shape), ap.dtype, name=f"sb_{name}")
                nc.sync.dma_start(t_, ap)
                sb[name] = t_

            # persistent state
            HX = cpool.tile([KG, NSLOT * BC], RD, name="HX")
            CTa = cpool.tile([H, 2, HB], RD, name="CTa")   # [c | tg] half A
            CTb = cpool.tile([H, 2, HB], RD, name="CTb")
            H1 = cpool.tile([H, BC], RD, name="H1")
            H2 = cpool.tile([H, BC], RD, name="H2")
            I32 = mybir.dt.int32
            Ma = [cpool.tile([H, HB], I32, name=f"Ma{r}") for r in range(2)]
            Mb = [cpool.tile([H, HB], I32, name=f"Mb{r}") for r in range(2)]
            nc.vector.memset(CTa, 0.0)
            nc.gpsimd.memset(CTb, 0.0)
            nc.vector.memset(H1, 0.0)
            nc.vector.memset(H2, 0.0)
            nc.gpsimd.memset(HX.bitcast(mybir.dt.uint16), 0)

            # x chunks 0 and 1 (steps 0..23); row KG-1 gets the ones row
            for b_ in range(2):
                nc.sync.dma_start(
                    HX[H:KG, b_ * CHUNK * BC:(b_ + 1) * CHUNK * BC],
                    ins["XD"][:, b_ * CHUNK * BC:(b_ + 1) * CHUNK * BC])

            def mm(out, lhsT, rhs, start=True, stop=True):
                nc.tensor.matmul(out, lhsT, rhs, start=start, stop=stop)

            # ---- encoder MLP -> h0 into slot 0 ----------------------------
            nc.sync.dma_start(HX[0:SD, 0:BC], ins["xTh"])
            ex = wpool.tile([SD, BC], F32R, name="ex")
            nc.sync.dma_start(ex, ins["xTr"])
            ez1p = ppool.tile([128, 512], F32, name="ez1p", tag="ps")
            for c in range(2):
                mm(ez1p[:, 256 * c:256 * (c + 1)],
                   sb["we1T"][:, 128 * c:128 * (c + 1)], ex)
            ez1 = wpool.tile([128, 512], F32R, name="ez1")
            for c in range(2):
                nc.scalar.activation(ez1[:, 256 * c:256 * (c + 1)],
                                     ez1p[:, 256 * c:256 * (c + 1)],
                                     AF.Tanh, bias=sb["be1"][:, c:c + 1])
            ez2p = ppool.tile([128, 512], F32, name="ez2p", tag="ps")
            for c in range(2):
                for k in range(2):
                    mm(ez2p[:, 256 * c:256 * (c + 1)],
                       sb[f"we2Tk{k}"][:, 128 * c:128 * (c + 1)],
                       ez1[:, 256 * k:256 * (k + 1)],
                       start=(k == 0), stop=(k == 1))
            ez2 = wpool.tile([128, 512], F32R, name="ez2")
            for c in range(2):
                nc.scalar.activation(ez2[:, 256 * c:256 * (c + 1)],
                                     ez2p[:, 256 * c:256 * (c + 1)],
                                     AF.Tanh, bias=sb["be2"][:, c:c + 1])
            eh0p = ppool.tile([64, 256], F32, name="eh0p", tag="ps")
            for k in range(2):
                mm(eh0p, sb[f"we3Tk{k}"], ez2[:, 256 * k:256 * (k + 1)],
                   start=(k == 0), stop=(k == 1))
            eh0 = wpool.tile([64, 256], RD, name="eh0")
            nc.scalar.activation(eh0, eh0p, AF.Identity, bias=sb["be3"][:, 0:1])
            nc.sync.dma_start(HX[SD:H, 0:BC], eh0)

            # ---- LSTM over T steps ----------------------------------------
            for t in range(cfg.tstop):
                base = (t % NSLOT) * BC
                nbase = ((t + 1) % NSLOT) * BC
                loA = int(lo_t[0][min(t, T)])
                loB = int(lo_t[1][min(t, T)])
                lvA = HB - loA
                lvB = HB - loB
                if lvA <= 0 and lvB <= 0:
                    break
                if t % CHUNK == 0 and t > 0:
                    b_ = t // CHUNK + 1
                    if b_ * CHUNK < T:
                        c0 = (b_ * CHUNK % NSLOT) * BC
                        n_ = min(CHUNK, T - b_ * CHUNK)
                        nc.sync.dma_start(
                            HX[H:KG, c0:c0 + n_ * BC],
                            ins["XD"][:, b_ * CHUNK * BC:(b_ * CHUNK + n_) * BC])

                # ---- half A: matmuls + ACT + DVE tail ---------------------
                if lvA > 0:
                    rhsA = HX[0:KG, base + loA:base + HB]
                    gpA = ppool.tile([H, 4, HB], F32, name="gpA", tag="psA")
                    for c in range(4):
                        mm(gpA[:, c, loA:HB], sb["wg_all"][:, H * c:H * (c + 1)], rhsA)
                if lvB > 0:
                    rhsB = HX[0:KG, base + HB + loB:base + 2 * HB]
                    gpB = ppool.tile([H, 4, HB], F32, name="gpB", tag="psB")
                    for c in range(4):
                        mm(gpB[:, c, loB:HB], sb["wg_all"][:, H * c:H * (c + 1)], rhsB)

                # sigma split: F,I,G on the chain; O off-chain (h needs it late)
                if lvA > 0:
                    SA = wpool.tile([H, 4, HB], RD, name="SA")
                    nc.scalar.activation(SA[:, 0:3, loA:HB], gpA[:, 0:3, loA:HB], AF.Sigmoid)
                if lvB > 0:
                    SB = wpool.tile([H, 4, HB], RD, name="SB")
                    nc.scalar.activation(SB[:, 0:3, loB:HB], gpB[:, 0:3, loB:HB], AF.Sigmoid)
                if lvA > 0:
                    nc.scalar.activation(SA[:, 3, loA:HB], gpA[:, 3, loA:HB], AF.Sigmoid)
                if lvB > 0:
                    nc.scalar.activation(SB[:, 3, loB:HB], gpB[:, 3, loB:HB], AF.Sigmoid)

                # DVE queue: [A: tg,Tt,add] [B: tg,Tt,add] [hA] [hB] so the
                # other half's ops fill the tanh-wait hole.
                if lvA > 0:
                    nc.vector.tensor_scalar(CTa[:, 1, loA:HB], SA[:, 2, loA:HB],
                                            2.0, -1.0, op0=ALU.mult, op1=ALU.add)
                    TtA = wpool.tile([H, 2, HB], RD, name="TtA")
                    nc.vector.tensor_tensor(TtA[:, :, loA:HB], SA[:, 0:2, loA:HB],
                                            CTa[:, :, loA:HB], op=ALU.mult)
                    nc.vector.tensor_tensor(CTa[:, 0, loA:HB], TtA[:, 0, loA:HB],
                                            TtA[:, 1, loA:HB], op=ALU.add)
                if lvB > 0:
                    nc.vector.tensor_scalar(CTb[:, 1, loB:HB], SB[:, 2, loB:HB],
                                            2.0, -1.0, op0=ALU.mult, op1=ALU.add)
                    TtB = wpool.tile([H, 2, HB], RD, name="TtB")
                    nc.vector.tensor_tensor(TtB[:, :, loB:HB], SB[:, 0:2, loB:HB],
                                            CTb[:, :, loB:HB], op=ALU.mult)
                    nc.vector.tensor_tensor(CTb[:, 0, loB:HB], TtB[:, 0, loB:HB],
                                            TtB[:, 1, loB:HB], op=ALU.add)
                if lvA > 0:
                    TCa = wpool.tile([H, HB], RD, name="TCa")
                    nc.scalar.activation(TCa[:, loA:HB], CTa[:, 0, loA:HB], AF.Tanh)
                    nc.vector.tensor_tensor(HX[0:H, nbase + loA:nbase + HB],
                                            SA[:, 3, loA:HB], TCa[:, loA:HB],
                                            op=ALU.mult)
                if lvB > 0:
                    TCb = wpool.tile([H, HB], RD, name="TCb")
                    nc.scalar.activation(TCb[:, loB:HB], CTb[:, 0, loB:HB], AF.Tanh)
                    nc.vector.tensor_tensor(HX[0:H, nbase + HB + loB:nbase + 2 * HB],
                                            SB[:, 3, loB:HB], TCb[:, loB:HB],
                                            op=ALU.mult)

                # ---- captures (masks + predicated copies) -----------------
                if t >= lmin - 2:
                    for h_, (M, off) in enumerate([(Ma, 0), (Mb, HB)]):
                        v2 = t + 2
                        if v2 <= T and whi_t[h_][v2] > wlo_t[h_][v2]:
                            a, b2 = int(wlo_t[h_][v2]), int(whi_t[h_][v2])
                            eng = nc.gpsimd
                            eng.tensor_scalar(M[t % 2][:, a:b2],
                                              sb["LENS"][:, off + a:off + b2],
                                              float(v2), None, op0=ALU.is_equal)
                            nc.vector.copy_predicated(
                                H2[:, off + a:off + b2], M[t % 2][:, a:b2],
                                HX[0:H, nbase + off + a:nbase + off + b2])
                        v1 = t + 1
                        if v1 >= lmin and whi_t[h_][v1] > wlo_t[h_][v1]:
                            a, b1 = int(wlo_t[h_][v1]), int(whi_t[h_][v1])
                            nc.vector.copy_predicated(
                                H1[:, off + a:off + b1], M[(t + 1) % 2][:, a:b1],
                                HX[0:H, nbase + off + a:nbase + off + b1])

            # ---- dec_in = d1*H1 + d2*H2 -----------------------------------
            U1 = wpool.tile([H, BC], F32, name="U1")
            nc.vector.tensor_tensor(U1, sb["d1"], H1, op=ALU.mult)
            U2 = wpool.tile([H, BC], F32, name="U2")
            nc.vector.tensor_tensor(U2, sb["d2"], H2, op=ALU.mult)
            DI = wpool.tile([H, BC], F32R, name="DI")
            nc.vector.tensor_tensor(DI, U1, U2, op=ALU.add)

            # ---- decoder MLP ----------------------------------------------
            CH1 = [(0, 128), (128, 128), (256, 32)]
            dz1p = ppool.tile([128, 768], F32, name="dz1p", tag="ps")
            for c, (off, m_) in enumerate(CH1):
                mm(dz1p[0:m_, 256 * c:256 * c + BC], sb["wd1T"][:, off:off + m_], DI)
            dz1 = wpool.tile([128, 768], F32R, name="dz1")
            for c, (off, m_) in enumerate(CH1):
                nc.scalar.activation(dz1[0:m_, 256 * c:256 * c + BC],
                                     dz1p[0:m_, 256 * c:256 * c + BC],
                                     AF.Tanh, bias=sb["bd1"][0:m_, c:c + 1])
            dz2p = ppool.tile([128, 768], F32, name="dz2p", tag="ps")
            for c, (off, m_) in enumerate(CH1):
                for k, (koff, km) in enumerate(CH1):
                    mm(dz2p[0:m_, 256 * c:256 * c + BC],
                       sb[f"wd2Tk{k}"][0:km, off:off + m_],
                       dz1[0:km, 256 * k:256 * k + BC],
                       start=(k == 0), stop=(k == 2))
            dz2 = wpool.tile([128, 768], F32R, name="dz2")
            for c, (off, m_) in enumerate(CH1):
                nc.scalar.activation(dz2[0:m_, 256 * c:256 * c + BC],
                                     dz2p[0:m_, 256 * c:256 * c + BC],
                                     AF.Tanh, bias=sb["bd2"][0:m_, c:c + 1])
            dz3p = ppool.tile([SD, 256], F32, name="dz3p", tag="ps")
            for k, (koff, km) in enumerate(CH1):
                mm(dz3p, sb[f"wd3Tk{k}"][0:km, :],
                   dz2[0:km, 256 * k:256 * k + BC],
                   start=(k == 0), stop=(k == 2))
            OUT = wpool.tile([SD, BC], F32, name="OUT")
            nc.scalar.activation(OUT, dz3p, AF.Identity, bias=sb["bd3"][:, 0:1])
            nc.sync.dma_start(out_dram, OUT)

    nc.compile()
    return nc


# --------------------------------------------------------------------------- #
# entry point
# --------------------------------------------------------------------------- #

def kernel(**inputs) -> np.ndarray:
    cfg = Cfg()
    in_maps, perms, meta = host_prep(inputs, cfg)
    nc = build_nc(cfg, meta)
    res = run_bass_kernel_spmd(nc, in_maps, core_ids=list(range(NCORES)))
    out = np.empty((B, SD), np.float32)
    for k in range(NCORES):
        out[perms[k]] = res.results[k]["out"].T
    return out
